# revision 4
# baseline (speedup 1.0000x reference)
"""CPGCN (2-layer GCN + two FC heads) on 8 Trainium2 NeuronCores.

Sharding: destination nodes are sharded across the 8 cores (6250 each,
degree-sorted within a core and padded to 6272 = 49 tiles of 128).
Each core computes xw = x @ W1 for its own nodes, scales by dinv
(symmetric GCN norm), all-gathers the scaled hidden table, then gathers
messages for its own destinations via indirect DMA using a padded-CSR
slot layout (host-built, shared per-tile K schedule across cores),
multiplies per-slot weights, reduces, applies bias/relu, repeats for
layer 2, and applies the two FC heads on device.  Host only does integer
index/layout work and the final row unpermutation.
"""
import sys
import numpy as np

sys.path.insert(0, "/opt/trn_rl_repo")

P = 128
N_NODES = 50000
N_CORES = 8
NODES_PER_CORE = N_NODES // N_CORES              # 6250
T = (NODES_PER_CORE + P - 1) // P                # 49 tiles
NP_PAD = T * P                                   # 6272
V_TAB = N_CORES * NP_PAD                         # 50176
NFEAT, NHID = 512, 64
NCLS = 48                                        # 16 + 32 concat
KB = NFEAT // P                                  # 4 contraction chunks


def _preprocess(edge_index, edge_weight):
    """Integer/layout-only prep: permutation, padded-CSR slots (single CSR,
    int32 indices into the global device-ordered table)."""
    row = np.asarray(edge_index[0], dtype=np.int64)
    col = np.asarray(edge_index[1], dtype=np.int64)
    w = np.asarray(edge_weight, dtype=np.float32)
    loops = np.arange(N_NODES, dtype=np.int64)
    row = np.concatenate([row, loops])
    col = np.concatenate([col, loops])
    w = np.concatenate([w, np.ones(N_NODES, np.float32)])

    indeg = np.bincount(col, minlength=N_NODES)
    perm = np.full((N_CORES, NP_PAD), -1, np.int64)
    pos = np.empty(N_NODES, np.int64)
    for c in range(N_CORES):
        nodes = np.arange(c * NODES_PER_CORE, (c + 1) * NODES_PER_CORE)
        order = nodes[np.argsort(-indeg[nodes], kind="stable")]
        perm[c, :NODES_PER_CORE] = order
        pos[order] = c * NP_PAD + np.arange(NODES_PER_CORE)

    src_pos = pos[row]
    dst_pos = pos[col]
    dst_core = dst_pos // NP_PAD
    dst_local = dst_pos % NP_PAD
    dst_tile = dst_local // P
    dst_part = dst_local % P

    counts = np.zeros((N_CORES, T, P), np.int64)
    np.add.at(counts, (dst_core, dst_tile, dst_part), 1)
    K = counts.max(axis=(0, 2))                  # [T] shared K schedule
    CT = int(K.sum())
    col_off = np.concatenate([[0], np.cumsum(K)])[:-1]

    idx_slots = np.zeros((N_CORES, P, CT), np.int32)   # pad -> row 0 (w=0)
    w_slots = np.zeros((N_CORES, P, CT), np.float32)
    order = np.lexsort((dst_part, dst_tile, dst_core))
    oc, ot, op_ = dst_core[order], dst_tile[order], dst_part[order]
    osrc, ow = src_pos[order], w[order]
    grp = (oc * T + ot) * P + op_
    first = np.r_[True, grp[1:] != grp[:-1]]
    gstart = np.flatnonzero(first)
    glen = np.diff(np.r_[gstart, len(grp)])
    krank = np.arange(len(grp)) - np.repeat(gstart, glen)
    colpos = col_off[ot] + krank
    idx_slots[oc, op_, colpos] = osrc.astype(np.int32)
    w_slots[oc, op_, colpos] = ow

    # dummy nodes (perm == -1): give one unit slot so deg=1 -> dinv=1 (finite);
    # their x rows are zero so table rows stay zero.
    for c in range(N_CORES):
        dummy_local = np.flatnonzero(perm[c] < 0)
        if len(dummy_local):
            dt_, dp_ = dummy_local // P, dummy_local % P
            w_slots[c, dp_, col_off[dt_]] = np.where(
                w_slots[c, dp_, col_off[dt_]] == 0, 1.0, w_slots[c, dp_, col_off[dt_]])
    return idx_slots, w_slots, K, col_off, CT, perm, pos


def _build_nc(K, col_off, CT):
    import concourse.bacc as bacc
    import concourse.tile as tile
    import concourse.mybir as mybir
    from concourse.bass import IndirectOffsetOnAxis

    dt = mybir.dt
    op = mybir.AluOpType

    nc = bacc.Bacc(None, target_bir_lowering=False)
    xT = nc.dram_tensor("xT", [T, KB, P, P], dt.float32, kind="ExternalInput")
    W1 = nc.dram_tensor("W1", [KB, P, NHID], dt.float32, kind="ExternalInput")
    W2 = nc.dram_tensor("W2", [NHID, NHID], dt.float32, kind="ExternalInput")
    fcW = nc.dram_tensor("fcW", [NHID, NCLS], dt.float32, kind="ExternalInput")
    b1r = nc.dram_tensor("b1r", [P, NHID], dt.float32, kind="ExternalInput")
    b2r = nc.dram_tensor("b2r", [P, NHID], dt.float32, kind="ExternalInput")
    fcbr = nc.dram_tensor("fcbr", [P, NCLS], dt.float32, kind="ExternalInput")
    iden = nc.dram_tensor("iden", [P, P], dt.float32, kind="ExternalInput")
    idxs = nc.dram_tensor("idxs", [P, CT], dt.int32, kind="ExternalInput")
    wsl = nc.dram_tensor("wsl", [P, CT], dt.float32, kind="ExternalInput")
    out = nc.dram_tensor("out", [NP_PAD, NCLS], dt.float32, kind="ExternalOutput")

    with tile.TileContext(nc) as tc:
        with (
            tc.tile_pool(name="const", bufs=1) as cpool,
            tc.tile_pool(name="work", bufs=3) as pool,
            tc.tile_pool(name="gath", bufs=2) as gpool,
            tc.tile_pool(name="psum", bufs=2, space="PSUM") as ppool,
            tc.tile_pool(name="dram", bufs=1, space="DRAM") as dram,
        ):
            # resident constants
            w_res = cpool.tile([P, CT], dt.float32)
            idx_res = cpool.tile([P, CT], dt.int32)
            nc.sync.dma_start(w_res[:], wsl[:])
            nc.sync.dma_start(idx_res[:], idxs[:])
            W1_sb = [cpool.tile([P, NHID], dt.float32, name=f"w1_{kb}", tag=f"w1_{kb}")
                     for kb in range(KB)]
            for kb in range(KB):
                nc.sync.dma_start(W1_sb[kb][:], W1[kb])
            W2_sb = cpool.tile([NHID, NHID], dt.float32)
            nc.sync.dma_start(W2_sb[:], W2[:])
            fcW_sb = cpool.tile([NHID, NCLS], dt.float32)
            nc.sync.dma_start(fcW_sb[:], fcW[:])
            b1_sb = cpool.tile([P, NHID], dt.float32)
            nc.sync.dma_start(b1_sb[:], b1r[:])
            b2_sb = cpool.tile([P, NHID], dt.float32)
            nc.sync.dma_start(b2_sb[:], b2r[:])
            fcb_sb = cpool.tile([P, NCLS], dt.float32)
            nc.sync.dma_start(fcb_sb[:], fcbr[:])
            iden_sb = cpool.tile([P, P], dt.float32)
            nc.sync.dma_start(iden_sb[:], iden[:])
            dinv_sb = cpool.tile([P, T], dt.float32)

            agin1 = dram.tile([NP_PAD, NHID], dt.float32)
            agout1 = dram.tile([V_TAB, NHID], dt.float32, addr_space="Shared")
            agin2 = dram.tile([NP_PAD, NHID], dt.float32)
            agout2 = dram.tile([V_TAB, NHID], dt.float32, addr_space="Shared")

            # ---- phase A: deg/dinv + xws ----
            for t in range(T):
                o, k = int(col_off[t]), int(K[t])
                deg = pool.tile([P, 1], dt.float32, tag="deg")
                nc.vector.tensor_reduce(
                    out=deg[:], in_=w_res[:, o:o + k],
                    axis=mybir.AxisListType.X, op=op.add)
                nc.scalar.activation(
                    out=deg[:], in_=deg[:],
                    func=mybir.ActivationFunctionType.Sqrt)
                nc.vector.reciprocal(out=dinv_sb[:, t:t + 1], in_=deg[:])
                pxw = ppool.tile([P, NHID], dt.float32, tag="pxw")
                for kb in range(KB):
                    xt_t = pool.tile([P, P], dt.float32, tag="xt")
                    nc.sync.dma_start(xt_t[:], xT[t, kb])
                    nc.tensor.matmul(pxw[:], lhsT=xt_t[:], rhs=W1_sb[kb][:],
                                     start=(kb == 0), stop=(kb == KB - 1))
                xws = pool.tile([P, NHID], dt.float32, tag="xws")
                nc.vector.tensor_scalar(
                    out=xws[:], in0=pxw[:], scalar1=dinv_sb[:, t:t + 1],
                    scalar2=None, op0=op.mult)
                nc.sync.dma_start(agin1[t * P:(t + 1) * P, :], xws[:])

            # ---- phase B: all-gather layer-1 table ----
            nc.gpsimd.collective_compute(
                "AllGather", op.bypass,
                replica_groups=[list(range(N_CORES))],
                ins=[agin1[:].opt()], outs=[agout1[:].opt()])

            def propagate(agout, t, bias_sb, relu):
                o, k = int(col_off[t]), int(K[t])
                g = gpool.tile([P, k * NHID], dt.float32, tag="g")
                for j in range(k):
                    nc.gpsimd.indirect_dma_start(
                        out=g[:, j * NHID:(j + 1) * NHID], out_offset=None,
                        in_=agout[:],
                        in_offset=IndirectOffsetOnAxis(
                            ap=idx_res[:, o + j:o + j + 1], axis=0))
                g3 = g[:].rearrange("p (k d) -> p k d", d=NHID)
                wb = w_res[:, o:o + k].unsqueeze(2).to_broadcast([P, k, NHID])
                nc.vector.tensor_tensor(out=g3, in0=g3, in1=wb, op=op.mult)
                red = pool.tile([P, NHID], dt.float32, tag="red")
                nc.vector.tensor_reduce(
                    out=red[:], in_=g[:].rearrange("p (k d) -> p d k", d=NHID),
                    axis=mybir.AxisListType.X, op=op.add)
                # h = (red * dinv + bias) [relu]
                nc.vector.tensor_scalar(
                    out=red[:], in0=red[:], scalar1=dinv_sb[:, t:t + 1],
                    scalar2=None, op0=op.mult)
                nc.vector.tensor_tensor(out=red[:], in0=red[:], in1=bias_sb[:], op=op.add)
                if relu:
                    nc.vector.tensor_scalar(
                        out=red[:], in0=red[:], scalar1=0.0, scalar2=None, op0=op.max)
                # transpose -> [64, 128]
                pT = ppool.tile([NHID, P], dt.float32, tag="pT")
                nc.tensor.transpose(pT[:], red[:], iden_sb[:])
                hT = pool.tile([NHID, P], dt.float32, tag="hT")
                nc.vector.tensor_copy(out=hT[:], in_=pT[:])
                return hT

            # ---- phase C: layer-1 propagate + W2 -> hs ----
            for t in range(T):
                hT = propagate(agout1, t, b1_sb, relu=True)
                pxw2 = ppool.tile([P, NHID], dt.float32, tag="pxw2")
                nc.tensor.matmul(pxw2[:], lhsT=hT[:], rhs=W2_sb[:],
                                 start=True, stop=True)
                hs = pool.tile([P, NHID], dt.float32, tag="hs")
                nc.vector.tensor_scalar(
                    out=hs[:], in0=pxw2[:], scalar1=dinv_sb[:, t:t + 1],
                    scalar2=None, op0=op.mult)
                nc.sync.dma_start(agin2[t * P:(t + 1) * P, :], hs[:])

            # ---- phase D: all-gather layer-2 table ----
            nc.gpsimd.collective_compute(
                "AllGather", op.bypass,
                replica_groups=[list(range(N_CORES))],
                ins=[agin2[:].opt()], outs=[agout2[:].opt()])

            # ---- phase E: layer-2 propagate + FC heads ----
            for t in range(T):
                h2T = propagate(agout2, t, b2_sb, relu=False)
                po = ppool.tile([P, NCLS], dt.float32, tag="po")
                nc.tensor.matmul(po[:], lhsT=h2T[:], rhs=fcW_sb[:],
                                 start=True, stop=True)
                ot_ = pool.tile([P, NCLS], dt.float32, tag="ot")
                nc.vector.tensor_tensor(out=ot_[:], in0=po[:], in1=fcb_sb[:], op=op.add)
                nc.sync.dma_start(out[t * P:(t + 1) * P, :], ot_[:])

    nc.finalize()
    return nc


def kernel(x, edge_index, edge_weight, W1, b1, W2, b2, fcW1, fcb1, fcW2, fcb2):
    from concourse.bass_utils import run_bass_kernel_spmd

    x = np.asarray(x, np.float32)
    W1 = np.asarray(W1, np.float32)
    W2 = np.asarray(W2, np.float32)
    b1 = np.asarray(b1, np.float32)
    b2 = np.asarray(b2, np.float32)
    fcW = np.concatenate([np.asarray(fcW1, np.float32), np.asarray(fcW2, np.float32)], axis=1)
    fcb = np.concatenate([np.asarray(fcb1, np.float32), np.asarray(fcb2, np.float32)])

    idx_slots, w_slots, K, col_off, CT, perm, pos = _preprocess(edge_index, edge_weight)

    nc = _build_nc(K, col_off, CT)

    W1_in = np.ascontiguousarray(W1.reshape(KB, P, NHID))
    b1r = np.tile(b1[None, :], (P, 1)).astype(np.float32)
    b2r = np.tile(b2[None, :], (P, 1)).astype(np.float32)
    fcbr = np.tile(fcb[None, :], (P, 1)).astype(np.float32)
    iden = np.eye(P, dtype=np.float32)

    in_maps = []
    for c in range(N_CORES):
        xp = np.zeros((NP_PAD, NFEAT), np.float32)
        valid = perm[c] >= 0
        xp[valid] = x[perm[c][valid]]
        # xT[t, kb] = xp[t*128:(t+1)*128, kb*128:(kb+1)*128].T
        xT = np.ascontiguousarray(
            xp.reshape(T, P, KB, P).transpose(0, 2, 3, 1))
        in_maps.append({
            "xT": xT, "W1": W1_in, "W2": W2, "fcW": fcW,
            "b1r": b1r, "b2r": b2r, "fcbr": fcbr, "iden": iden,
            "idxs": np.ascontiguousarray(idx_slots[c]),
            "wsl": np.ascontiguousarray(w_slots[c]),
        })

    res = run_bass_kernel_spmd(nc, in_maps, core_ids=list(range(N_CORES)))
    full = np.concatenate([res.results[c]["out"] for c in range(N_CORES)], axis=0)
    full = full[pos]                              # unpermute to original node order
    return full[:, :16].copy(), full[:, 16:].copy()
